# revision 2
# baseline (speedup 1.0000x reference)
"""Trainium2 Bass kernel for nn_CoordsToNRF.

out[b, p] = atom_nc[b, p] * (AU2KCALMOLA / MAX_NRF) / ||coords[b, I[p]] - coords[b, J[p]]||^2

Strategy (pure data parallel over batch, 8 cores x 128 batches):
  - Layout: batch on partitions, pairs on the free dim ([128, 8128] per core).
  - The pair gather+subtract is one matmul per xyz component:
        D_c = CT_c.T @ S            (TensorEngine, PSUM out)
    where CT_c [atom, batch] are transposed coords (built on-chip with PE
    transposes) and S [atom, pairs] is the static +1/-1 tril selection matrix.
  - ScalarE squares with a folded scale: sq_c = (s*D_c)^2, s = 1/sqrt(K), so
    r2' = r2/K and the final multiply needs no extra constant.
  - VectorE: r2' = sq0+sq1+sq2, inv = 1/r2', out = atom_nc * inv.
"""

import sys

for _p in ("/opt/trn_rl_repo",):
    if _p not in sys.path:
        sys.path.insert(0, _p)

import numpy as np
from contextlib import ExitStack

import concourse.bass as bass
import concourse.bacc as bacc
import concourse.tile as tile
from concourse import mybir
from concourse.bass_utils import run_bass_kernel_spmd

F32 = mybir.dt.float32

N_ATOMS = 128
NC2 = N_ATOMS * (N_ATOMS - 1) // 2  # 8128
BATCH = 1024
N_CORES = 8
BPC = BATCH // N_CORES  # 128 batches per core

AU2KCALMOLA = 627.5095 * 0.529177
MAX_NRF = 13036.0
K_CONST = AU2KCALMOLA / MAX_NRF
SQ_SCALE = float(1.0 / np.sqrt(K_CONST))  # fold K into the square

GROUP = 512  # pairs per group (one PSUM bank of fp32)
GROUPS = [(g, min(GROUP, NC2 - g)) for g in range(0, NC2, GROUP)]

_I, _J = np.tril_indices(N_ATOMS, -1)


def _build_smat() -> np.ndarray:
    s = np.zeros((N_ATOMS, NC2), dtype=np.float32)
    p = np.arange(NC2)
    s[_I, p] = 1.0
    s[_J, p] = -1.0
    return s


def _build_program():
    nc = bacc.Bacc("TRN2", target_bir_lowering=False, debug=False)

    coords_d = nc.dram_tensor("coords", [BPC, N_ATOMS * 3], F32, kind="ExternalInput")
    anc_d = nc.dram_tensor("atom_nc", [BPC, NC2], F32, kind="ExternalInput")
    smat_d = nc.dram_tensor("smat", [N_ATOMS, NC2], F32, kind="ExternalInput")
    ident_d = nc.dram_tensor("ident", [128, 128], F32, kind="ExternalInput")
    out_d = nc.dram_tensor("out", [BPC, NC2], F32, kind="ExternalOutput")

    with tile.TileContext(nc) as tc, ExitStack() as ctx:
        const = ctx.enter_context(tc.tile_pool(name="const", bufs=1))
        work = ctx.enter_context(tc.tile_pool(name="work", bufs=3))
        outp = ctx.enter_context(tc.tile_pool(name="outp", bufs=3))
        ps_t = ctx.enter_context(tc.tile_pool(name="ps_t", bufs=1, space="PSUM"))
        ps_d = ctx.enter_context(tc.tile_pool(name="ps_d", bufs=2, space="PSUM"))

        # ---- constant/setup loads ----
        ident_sb = const.tile([128, 128], F32)
        nc.sync.dma_start(ident_sb[:], ident_d[:, :])

        coords_sb = const.tile([BPC, N_ATOMS, 3], F32)
        nc.sync.dma_start(coords_sb[:], coords_d[:, :].rearrange("b (a c) -> b a c", c=3))

        smat_sb = const.tile([N_ATOMS, NC2], F32)
        anc_sb = const.tile([BPC, NC2], F32)
        N_LOAD_CHUNKS = 4
        cw = NC2 // N_LOAD_CHUNKS  # 2032
        for i in range(N_LOAD_CHUNKS):
            c0 = i * cw
            c1 = NC2 if i == N_LOAD_CHUNKS - 1 else (i + 1) * cw
            nc.sync.dma_start(smat_sb[:, c0:c1], smat_d[:, c0:c1])
            nc.sync.dma_start(anc_sb[:, c0:c1], anc_d[:, c0:c1])

        # ---- coords transpose: CT_c [atom, batch], c = 0,1,2 ----
        ct_sb = const.tile([N_ATOMS, 3, BPC], F32)
        for c in range(3):
            t_ps = ps_t.tile([128, 128], F32)
            nc.tensor.transpose(t_ps[:], coords_sb[:, :, c], ident_sb[:])
            nc.scalar.copy(ct_sb[:, c, :], t_ps[:])

        # ---- main loop over pair groups ----
        for gs, fd in GROUPS:
            d_ps = ps_d.tile([128, 3, GROUP], F32)
            for c in range(3):
                nc.tensor.matmul(
                    d_ps[:, c, :fd],
                    ct_sb[:, c, :],              # lhsT [atom, batch] fp32
                    smat_sb[:, gs:gs + fd],      # rhs  [atom, fd] fp32
                    start=True, stop=True,
                )
            sq = work.tile([128, 3, GROUP], F32)
            for c in range(3):
                nc.scalar.activation(
                    sq[:, c, :fd], d_ps[:, c, :fd],
                    mybir.ActivationFunctionType.Square,
                    bias=0.0, scale=SQ_SCALE,
                )
            r2 = work.tile([128, GROUP], F32, tag="r2")
            nc.vector.tensor_add(r2[:, :fd], sq[:, 0, :fd], sq[:, 1, :fd])
            nc.vector.tensor_add(r2[:, :fd], r2[:, :fd], sq[:, 2, :fd])
            nc.vector.reciprocal(r2[:, :fd], r2[:, :fd])
            o = outp.tile([128, GROUP], F32)
            nc.vector.tensor_mul(o[:, :fd], r2[:, :fd], anc_sb[:, gs:gs + fd])
            nc.sync.dma_start(out_d[:, gs:gs + fd], o[:, :fd])

    nc.compile()
    return nc


_CACHED = None


def _get_program():
    global _CACHED
    if _CACHED is None:
        _CACHED = _build_program()
    return _CACHED


def kernel(coords, atom_nc, _trace=False, _trace_kwargs=None):
    coords = np.ascontiguousarray(np.asarray(coords, dtype=np.float32))
    atom_nc = np.ascontiguousarray(np.asarray(atom_nc, dtype=np.float32))
    assert coords.shape == (BATCH, N_ATOMS, 3)
    assert atom_nc.shape == (BATCH, NC2)

    nc = _get_program()
    smat = _build_smat()
    ident = np.eye(128, dtype=np.float32)

    in_maps = []
    for core in range(N_CORES):
        b0 = core * BPC
        in_maps.append({
            "coords": coords[b0:b0 + BPC].reshape(BPC, N_ATOMS * 3).copy(),
            "atom_nc": atom_nc[b0:b0 + BPC],
            "smat": smat,
            "ident": ident,
        })

    kw = {}
    if _trace:
        kw["trace"] = True
        kw.update(_trace_kwargs or {})
    res = run_bass_kernel_spmd(nc, in_maps, core_ids=list(range(N_CORES)), **kw)
    out = np.concatenate([r["out"] for r in res.results], axis=0)
    if _trace:
        return out, res
    return out


if __name__ == "__main__":
    rng = np.random.default_rng(0)
    coords = (rng.standard_normal((BATCH, N_ATOMS, 3)) * 5.0).astype(np.float32)
    atom_nc = rng.uniform(1.0, 50.0, (BATCH, NC2)).astype(np.float32)
    out = kernel(coords, atom_nc)
    print(out.shape, out.dtype)


# revision 3
# speedup vs baseline: 1.5198x; 1.5198x over previous
"""Trainium2 Bass kernel for nn_CoordsToNRF.

out[b, p] = atom_nc[b, p] * (AU2KCALMOLA / MAX_NRF) / ||coords[b, I[p]] - coords[b, J[p]]||^2

Strategy (pure data parallel over batch, 8 cores x 128 batches):
  - Layout: batch on partitions, pairs on the free dim ([128, 8128] per core).
  - The pair gather+subtract runs on the TensorEngine: per xyz component,
        D_c = CT_c.T @ S
    with S [atom, pairs] the static +1/-1 tril selection matrix. For speed the
    matmuls are fp16 (1 cyc/row vs 4 for fp32) with an exact two-term split:
        C = C_hi + 2^-11 * C_lo,  C_hi = fp16(C), C_lo = fp16(2^11*(C - C_hi))
    The lo product uses S_lo = S * 2^-11 (exact in fp16), so both matmuls
    accumulate s.t. PSUM = D to ~2^-24 relative accuracy.
  - ScalarE squares all 3 planes in one activation with a folded scale
    s = 1/sqrt(K):  r2' = r2/K.
  - VectorE: two adds, then reciprocal_approx_fast (18-bit accurate).
  - GpSimd: final multiply by atom_nc (keeps VectorE off the critical path).
"""

import sys

for _p in ("/opt/trn_rl_repo",):
    if _p not in sys.path:
        sys.path.insert(0, _p)

import numpy as np
from contextlib import ExitStack

import concourse.bass as bass
import concourse.bacc as bacc
import concourse.tile as tile
from concourse import mybir
from concourse.bass_utils import run_bass_kernel_spmd

F32 = mybir.dt.float32
F16 = mybir.dt.float16

N_ATOMS = 128
NC2 = N_ATOMS * (N_ATOMS - 1) // 2  # 8128
BATCH = 1024
N_CORES = 8
BPC = BATCH // N_CORES  # 128 batches per core

AU2KCALMOLA = 627.5095 * 0.529177
MAX_NRF = 13036.0
K_CONST = AU2KCALMOLA / MAX_NRF
SQ_SCALE = float(1.0 / np.sqrt(K_CONST))  # fold K into the square
LO_SHIFT = 2.0 ** 11

GROUP = 512  # pairs per group (one PSUM bank of fp32)
GROUPS = [(g, min(GROUP, NC2 - g)) for g in range(0, NC2, GROUP)]
CHUNK = 2048  # input-load chunk = 4 groups
CHUNKS = [(c, min(CHUNK, NC2 - c)) for c in range(0, NC2, CHUNK)]

_I, _J = np.tril_indices(N_ATOMS, -1)


def _build_smat_f16() -> np.ndarray:
    s = np.zeros((N_ATOMS, NC2), dtype=np.float16)
    p = np.arange(NC2)
    s[_I, p] = 1.0
    s[_J, p] = -1.0
    return s


def _split_coords(coords32: np.ndarray):
    """coords32 [B, A*3] f32 -> (hi, lo) fp16 with C ~= hi + lo/2^11."""
    hi = coords32.astype(np.float16)
    lo = ((coords32 - hi.astype(np.float32)) * LO_SHIFT).astype(np.float16)
    return hi, lo


def _build_program():
    nc = bacc.Bacc("TRN2", target_bir_lowering=False, debug=False)

    chi_d = nc.dram_tensor("coords_hi", [BPC, N_ATOMS * 3], F16, kind="ExternalInput")
    clo_d = nc.dram_tensor("coords_lo", [BPC, N_ATOMS * 3], F16, kind="ExternalInput")
    anc_d = nc.dram_tensor("atom_nc", [BPC, NC2], F32, kind="ExternalInput")
    smat_d = nc.dram_tensor("smat", [N_ATOMS, NC2], F16, kind="ExternalInput")
    ident_d = nc.dram_tensor("ident", [128, 128], F16, kind="ExternalInput")
    out_d = nc.dram_tensor("out", [BPC, NC2], F32, kind="ExternalOutput")

    with tile.TileContext(nc) as tc, ExitStack() as ctx:
        const = ctx.enter_context(tc.tile_pool(name="const", bufs=1))
        work = ctx.enter_context(tc.tile_pool(name="work", bufs=3))
        outp = ctx.enter_context(tc.tile_pool(name="outp", bufs=3))
        ps_t = ctx.enter_context(tc.tile_pool(name="ps_t", bufs=1, space="PSUM"))
        ps_d = ctx.enter_context(tc.tile_pool(name="ps_d", bufs=2, space="PSUM"))

        # ---- constants / inputs ----
        ident_sb = const.tile([128, 128], F16)
        nc.sync.dma_start(ident_sb[:], ident_d[:, :])

        chi_sb = const.tile([BPC, N_ATOMS, 3], F16)
        clo_sb = const.tile([BPC, N_ATOMS, 3], F16)
        nc.sync.dma_start(chi_sb[:], chi_d[:, :].rearrange("b (a c) -> b a c", c=3))
        nc.sync.dma_start(clo_sb[:], clo_d[:, :].rearrange("b (a c) -> b a c", c=3))

        # chunked loads so early groups don't wait on the whole 8k columns
        smat_sb, slo_sb, anc_sb = [], [], []
        for ci, (c0, cw) in enumerate(CHUNKS):
            st = const.tile([N_ATOMS, cw], F16, tag=f"smat{ci}")
            nc.sync.dma_start(st[:], smat_d[:, c0:c0 + cw])
            smat_sb.append(st)
            at = const.tile([BPC, cw], F32, tag=f"anc{ci}")
            nc.sync.dma_start(at[:], anc_d[:, c0:c0 + cw])
            anc_sb.append(at)
            # S_lo = S * 2^-11 (exact in fp16)
            sl = const.tile([N_ATOMS, cw], F16, tag=f"slo{ci}")
            nc.vector.tensor_scalar_mul(sl[:], st[:], 1.0 / LO_SHIFT)
            slo_sb.append(sl)

        # ---- coords transposes: CT[hi/lo] [atom, 3, batch] fp16 ----
        cthi_sb = const.tile([N_ATOMS, 3, BPC], F16, tag="cthi")
        ctlo_sb = const.tile([N_ATOMS, 3, BPC], F16, tag="ctlo")
        for src, dst in ((chi_sb, cthi_sb), (clo_sb, ctlo_sb)):
            for c in range(3):
                t_ps = ps_t.tile([128, 128], F16)
                nc.tensor.transpose(t_ps[:], src[:, :, c], ident_sb[:])
                nc.scalar.copy(dst[:, c, :], t_ps[:])

        # ---- main loop over pair groups ----
        for gs, fd in GROUPS:
            ci, off = gs // CHUNK, gs % CHUNK
            d_ps = ps_d.tile([128, 3, GROUP], F32)
            for c in range(3):
                nc.tensor.matmul(
                    d_ps[:, c, :fd], cthi_sb[:, c, :],
                    smat_sb[ci][:, off:off + fd],
                    start=True, stop=False,
                )
                nc.tensor.matmul(
                    d_ps[:, c, :fd], ctlo_sb[:, c, :],
                    slo_sb[ci][:, off:off + fd],
                    start=False, stop=True,
                )
            # squares of all 3 planes in one activation (PSUM -> SBUF)
            sq = work.tile([128, 3, GROUP], F32)
            nc.scalar.activation(
                sq[:, :, :fd], d_ps[:, :, :fd],
                mybir.ActivationFunctionType.Square,
                bias=0.0, scale=SQ_SCALE,
            )
            r2 = work.tile([128, GROUP], F32, tag="r2")
            nc.vector.tensor_add(r2[:, :fd], sq[:, 0, :fd], sq[:, 1, :fd])
            nc.vector.tensor_add(r2[:, :fd], r2[:, :fd], sq[:, 2, :fd])
            inv = work.tile([128, GROUP], F32, tag="inv")
            nc.vector.reciprocal_approx_fast(inv[:, :fd], r2[:, :fd])
            o = outp.tile([128, GROUP], F32)
            nc.gpsimd.tensor_mul(o[:, :fd], inv[:, :fd], anc_sb[ci][:, off:off + fd])
            nc.sync.dma_start(out_d[:, gs:gs + fd], o[:, :fd])

    nc.compile()
    return nc


_CACHED = None


def _get_program():
    global _CACHED
    if _CACHED is None:
        _CACHED = _build_program()
    return _CACHED


def kernel(coords, atom_nc, _trace=False, _trace_kwargs=None):
    coords = np.ascontiguousarray(np.asarray(coords, dtype=np.float32))
    atom_nc = np.ascontiguousarray(np.asarray(atom_nc, dtype=np.float32))
    assert coords.shape == (BATCH, N_ATOMS, 3)
    assert atom_nc.shape == (BATCH, NC2)

    nc = _get_program()
    smat = _build_smat_f16()
    ident = np.eye(128, dtype=np.float16)
    chi, clo = _split_coords(coords.reshape(BATCH, N_ATOMS * 3))

    in_maps = []
    for core in range(N_CORES):
        b0 = core * BPC
        in_maps.append({
            "coords_hi": chi[b0:b0 + BPC],
            "coords_lo": clo[b0:b0 + BPC],
            "atom_nc": atom_nc[b0:b0 + BPC],
            "smat": smat,
            "ident": ident,
        })

    kw = {}
    if _trace:
        kw["trace"] = True
        kw.update(_trace_kwargs or {})
    res = run_bass_kernel_spmd(nc, in_maps, core_ids=list(range(N_CORES)), **kw)
    out = np.concatenate([r["out"] for r in res.results], axis=0)
    if _trace:
        return out, res
    return out


if __name__ == "__main__":
    rng = np.random.default_rng(0)
    coords = (rng.standard_normal((BATCH, N_ATOMS, 3)) * 5.0).astype(np.float32)
    atom_nc = rng.uniform(1.0, 50.0, (BATCH, NC2)).astype(np.float32)
    out = kernel(coords, atom_nc)
    print(out.shape, out.dtype)
